# revision 4
# baseline (speedup 1.0000x reference)
"""MoE routing kernel for Trainium2 (8 NeuronCores, SPMD expert-parallel).

Contract: kernel(**full_inputs) -> full output [B, S, H] float32.

Strategy
--------
- Host: compute the (tiny) gate + group-topk routing in numpy (bit-identical
  selection to the jax reference), build the per-(token,expert) combine
  weights, and dispatch: gather each expert's tokens into a padded,
  transposed bf16 buffer.  This is the "all-to-all by topk_idx" of the
  sharding hint, done at input-sharding time.
- Device (SPMD over 8 cores): core c holds experts (2c, 2c+1) and a 1/8
  column-slice of the shared expert.  Each core runs the SwiGLU MLP for its
  two experts over their gathered tokens (unweighted), plus its shared
  slice over all tokens, producing partial outputs in [H, tokens] layout.
- Host: scale per-expert outputs by routing weights, scatter-add over
  token indices, add the 8 shared partials, transpose back.

All matmuls run in bf16 with fp32 PSUM accumulation.
"""

import math

import numpy as np
import ml_dtypes

H = 2048          # hidden size
I = 1408          # intermediate per routed expert
E = 16            # routed experts
G = 4             # groups
TOPK_GROUP = 2
TOP_K = 6
N_SHARED = 2
SCALE_FACTOR = 2.5
SI = I * N_SHARED  # 2816 shared intermediate
N_CORES = 8
EXP_PER_CORE = E // N_CORES  # 2
S_SLICE_RAW = SI // N_CORES  # 352
S_SLICE = 384                # padded to 3*128
P = 128
BF16 = ml_dtypes.bfloat16

_COMPILED = {}  # (T, C_cap, w) -> (nc, meta)
_LAST = {}      # debug/profiling handle for test.py


def _gate_host(hs, gate_weight, bias):
    """numpy replica of reference._gate (verified bit-identical selection)."""
    T = hs.shape[0]
    logits = hs @ gate_weight.T                       # [T, E] fp32
    scores = 1.0 / (1.0 + np.exp(-logits))
    sfc = scores + bias[None, :]
    gs = sfc.reshape(T, G, E // G)
    gsort = np.sort(gs, axis=-1)
    group_scores = gsort[..., -1] + gsort[..., -2]
    group_idx = np.argsort(-group_scores, axis=-1, kind="stable")[:, :TOPK_GROUP]
    gmask = np.zeros((T, G), bool)
    gmask[np.arange(T)[:, None], group_idx] = True
    smask = np.repeat(gmask, E // G, axis=1)
    tmp = np.where(smask, sfc, 0.0)
    topk_idx = np.argsort(-tmp, axis=-1, kind="stable")[:, :TOP_K]
    topk_w = np.take_along_axis(scores, topk_idx, axis=1)
    topk_w = topk_w / (topk_w.sum(-1, keepdims=True) + 1e-20) * SCALE_FACTOR
    return topk_idx.astype(np.int32), topk_w.astype(np.float32)


def _build(T, C_cap, w):
    """Build + compile the SPMD Bass program.

    T     : total tokens (per core sees all of them for the shared slice)
    C_cap : per-expert gathered-token capacity (multiple of 4*w_... = 4w)
    w     : routed matmul free-dim slice width (C_cap = 4*w, w <= 512)
    """
    import concourse.mybir as mybir
    import concourse.tile as tile
    from concourse import bacc

    bf = mybir.dt.bfloat16
    f32 = mybir.dt.float32
    AF = mybir.ActivationFunctionType

    KH = H // P    # 16 contraction chunks over H
    MI = I // P    # 11 I chunks
    MH = H // P    # 16 output H chunks
    MS = S_SLICE // P  # 3
    NT = T // 512  # shared token slices
    NPAIR = C_cap // (2 * w)  # routed token blocks (each 2*w wide)
    assert C_cap == NPAIR * 2 * w

    nc = bacc.Bacc("TRN2", target_bir_lowering=False, debug=False,
                   num_devices=N_CORES)
    xs = nc.dram_tensor("xs", [H, T], bf, kind="ExternalInput")
    xg = nc.dram_tensor("xg", [H, EXP_PER_CORE * C_cap], bf, kind="ExternalInput")
    wg = nc.dram_tensor("wg", [H, EXP_PER_CORE * I], bf, kind="ExternalInput")
    wu = nc.dram_tensor("wu", [H, EXP_PER_CORE * I], bf, kind="ExternalInput")
    wd = nc.dram_tensor("wd", [I, EXP_PER_CORE * H], bf, kind="ExternalInput")
    sg = nc.dram_tensor("sg", [H, S_SLICE], bf, kind="ExternalInput")
    su = nc.dram_tensor("su", [H, S_SLICE], bf, kind="ExternalInput")
    sd = nc.dram_tensor("sd", [S_SLICE, H], bf, kind="ExternalInput")
    ye = nc.dram_tensor("ye", [H, EXP_PER_CORE * C_cap], bf, kind="ExternalOutput")
    ys = nc.dram_tensor("ys", [H, T], bf, kind="ExternalOutput")

    with tile.TileContext(nc) as tc:
        with (
            tc.tile_pool(name="xgp", bufs=KH + 4) as xgp,      # [128, 2w] bf16
            tc.tile_pool(name="xsp", bufs=KH + 4) as xsp,      # [128, 512] bf16
            tc.tile_pool(name="wst", bufs=12) as wst,          # [128, 128] weight stream
            tc.tile_pool(name="wdp", bufs=MI) as wdp,          # [128, 2H] resident down w
            tc.tile_pool(name="sdp", bufs=MS) as sdp,          # [128, H] resident shared down w
            tc.tile_pool(name="itp", bufs=4 * MI) as itp,      # [128, w] inter
            tc.tile_pool(name="sit", bufs=2 * MS + 2) as sit,  # [128, 512] shared inter
            tc.tile_pool(name="tmp", bufs=6) as tmp,           # silu temps
            tc.tile_pool(name="otp", bufs=6) as otp,           # psum->sbuf out (bf16)
            tc.tile_pool(name="pg", bufs=3, space="PSUM") as pgp,
            tc.tile_pool(name="pu", bufs=3, space="PSUM") as pup,
            tc.tile_pool(name="py", bufs=2, space="PSUM") as pyp,
        ):
            # ---------------- shared expert (column slice) ----------------
            sd_t = []
            for kk in range(MS):
                t = sdp.tile([P, H], bf, name=f"sdt{kk}", tag="sdt")
                nc.sync.dma_start(t[:], sd[kk * P:(kk + 1) * P, :])
                sd_t.append(t)

            for n in range(NT):
                c0 = n * 512
                xst = []
                for k in range(KH):
                    t = xsp.tile([P, 512], bf, name=f"xs_{n}_{k}", tag="xst")
                    nc.sync.dma_start(t[:], xs[k * P:(k + 1) * P, c0:c0 + 512])
                    xst.append(t)
                sint = []
                for m in range(MS):
                    psg = pgp.tile([P, 512], f32, name=f"psg_s{n}_{m}", tag="pg")
                    psu = pup.tile([P, 512], f32, name=f"psu_s{n}_{m}", tag="pu")
                    for k in range(KH):
                        sgt = wst.tile([P, P], bf, name=f"sgt{n}_{m}_{k}", tag="wst")
                        nc.sync.dma_start(
                            sgt[:], sg[k * P:(k + 1) * P, m * P:(m + 1) * P])
                        sut = wst.tile([P, P], bf, name=f"sut{n}_{m}_{k}", tag="wst")
                        nc.sync.dma_start(
                            sut[:], su[k * P:(k + 1) * P, m * P:(m + 1) * P])
                        nc.tensor.matmul(psg[:], sgt[:], xst[k][:],
                                         start=(k == 0), stop=(k == KH - 1))
                        nc.tensor.matmul(psu[:], sut[:], xst[k][:],
                                         start=(k == 0), stop=(k == KH - 1))
                    st = tmp.tile([P, 512], bf, name=f"st_s{n}_{m}", tag="tmp")
                    nc.scalar.activation(st[:], psg[:], AF.Silu)
                    it = sit.tile([P, 512], bf, name=f"si_{n}_{m}", tag="sit")
                    nc.vector.tensor_mul(it[:], st[:], psu[:])
                    sint.append(it)
                for M in range(MH):
                    psy = pyp.tile([P, 512], f32, name=f"psy_s{n}_{M}", tag="py")
                    for K in range(MS):
                        nc.tensor.matmul(psy[:], sd_t[K][:, M * P:(M + 1) * P],
                                         sint[K][:],
                                         start=(K == 0), stop=(K == MS - 1))
                    ot = otp.tile([P, 512], bf, name=f"ot_s{n}_{M}", tag="otp")
                    nc.vector.tensor_copy(ot[:], psy[:])
                    nc.sync.dma_start(ys[M * P:(M + 1) * P, c0:c0 + 512], ot[:])

            # ---------------- routed experts ----------------
            for s in range(EXP_PER_CORE):
                wd_t = []
                for K in range(MI):
                    t = wdp.tile([P, H], bf, name=f"wdt{s}_{K}", tag="wdt")
                    nc.sync.dma_start(
                        t[:], wd[K * P:(K + 1) * P, s * H:(s + 1) * H])
                    wd_t.append(t)
                for np_ in range(NPAIR):
                    b0 = s * C_cap + np_ * 2 * w
                    xgt = []
                    for k in range(KH):
                        t = xgp.tile([P, 2 * w], bf, name=f"xg{s}_{np_}_{k}",
                                     tag="xgt")
                        nc.sync.dma_start(
                            t[:], xg[k * P:(k + 1) * P, b0:b0 + 2 * w])
                        xgt.append(t)
                    inter = {}
                    for m in range(MI):
                        psg = [pgp.tile([P, 512], f32,
                                        name=f"psg{s}_{np_}_{m}_{j}", tag="pg")
                               for j in range(2)]
                        psu = [pup.tile([P, 512], f32,
                                        name=f"psu{s}_{np_}_{m}_{j}", tag="pu")
                               for j in range(2)]
                        for k in range(KH):
                            wgt = wst.tile([P, P], bf,
                                           name=f"wgt{s}_{np_}_{m}_{k}", tag="wst")
                            nc.sync.dma_start(
                                wgt[:], wg[k * P:(k + 1) * P,
                                           s * I + m * P:s * I + (m + 1) * P])
                            wut = wst.tile([P, P], bf,
                                           name=f"wut{s}_{np_}_{m}_{k}", tag="wst")
                            nc.sync.dma_start(
                                wut[:], wu[k * P:(k + 1) * P,
                                           s * I + m * P:s * I + (m + 1) * P])
                            for j in range(2):
                                nc.tensor.matmul(
                                    psg[j][:, :w], wgt[:],
                                    xgt[k][:, j * w:(j + 1) * w],
                                    start=(k == 0), stop=(k == KH - 1))
                                nc.tensor.matmul(
                                    psu[j][:, :w], wut[:],
                                    xgt[k][:, j * w:(j + 1) * w],
                                    start=(k == 0), stop=(k == KH - 1))
                        for j in range(2):
                            st = tmp.tile([P, 512], bf,
                                          name=f"st{s}_{np_}_{m}_{j}", tag="tmp")
                            nc.scalar.activation(st[:, :w], psg[j][:, :w], AF.Silu)
                            it = itp.tile([P, w], bf,
                                          name=f"it{s}_{np_}_{m}_{j}", tag="itp")
                            nc.vector.tensor_mul(it[:], st[:, :w], psu[j][:, :w])
                            inter[(m, j)] = it
                    for M in range(MH):
                        for j in range(2):
                            psy = pyp.tile([P, 512], f32,
                                           name=f"psy{s}_{np_}_{M}_{j}", tag="py")
                            for K in range(MI):
                                nc.tensor.matmul(
                                    psy[:, :w],
                                    wd_t[K][:, M * P:(M + 1) * P],
                                    inter[(K, j)][:],
                                    start=(K == 0), stop=(K == MI - 1))
                            ot = otp.tile([P, 512], bf,
                                          name=f"ot{s}_{np_}_{M}_{j}", tag="otp")
                            nc.vector.tensor_copy(ot[:, :w], psy[:, :w])
                            nc.sync.dma_start(
                                ye[M * P:(M + 1) * P,
                                   b0 + j * w:b0 + (j + 1) * w],
                                ot[:, :w])

    nc.compile()
    return nc


def _get_compiled(T, C_cap, w):
    key = (T, C_cap, w)
    if key not in _COMPILED:
        _COMPILED[key] = _build(T, C_cap, w)
    return _COMPILED[key]


def kernel(hidden_states, gate_weight, e_score_correction_bias,
           gate_proj, up_proj, down_proj,
           shared_gate_w, shared_up_w, shared_down_w):
    from concourse.bass_utils import run_bass_kernel_spmd

    hs = np.asarray(hidden_states, dtype=np.float32)
    B, S, Hh = hs.shape
    assert Hh == H
    hsf = np.ascontiguousarray(hs.reshape(-1, H))
    T = hsf.shape[0]
    gate_weight = np.asarray(gate_weight, np.float32)
    bias = np.asarray(e_score_correction_bias, np.float32)
    gate_proj = np.asarray(gate_proj, np.float32)
    up_proj = np.asarray(up_proj, np.float32)
    down_proj = np.asarray(down_proj, np.float32)
    shared_gate_w = np.asarray(shared_gate_w, np.float32)
    shared_up_w = np.asarray(shared_up_w, np.float32)
    shared_down_w = np.asarray(shared_down_w, np.float32)

    # ---- routing on host ----
    topk_idx, topk_w = _gate_host(hsf, gate_weight, bias)
    comb = np.zeros((T, E), np.float32)
    np.add.at(comb, (np.arange(T)[:, None], topk_idx), topk_w)
    sel = np.zeros((T, E), bool)
    sel[np.arange(T)[:, None], topk_idx] = True
    idx_e = [np.nonzero(sel[:, e])[0] for e in range(E)]
    counts = np.array([len(ix) for ix in idx_e])

    maxc = int(counts.max())
    w = min(512, ((maxc + 4 * 8 - 1) // (4 * 8)) * 8)  # ceil(maxc/4) to mult of 8
    while 4 * w < maxc:  # safety for very skewed counts
        w = min(512, w + 8)
    if 4 * w < maxc:
        # beyond single-npair-pair capacity: fall back to full T capacity
        w = 512
    C_cap = 4 * w
    n_rounds = 1
    if maxc > C_cap:  # extremely skewed: process in multiple rounds
        n_rounds = math.ceil(maxc / C_cap)

    # ---- host-side dispatch (shard + transpose + bf16 cast) ----
    xsT = np.ascontiguousarray(hsf.T).astype(BF16)          # [H, T]
    xg_all = np.zeros((E, H, C_cap * n_rounds), BF16)
    for e in range(E):
        xg_all[e][:, :counts[e]] = xsT[:, idx_e[e]]

    in_maps = []
    for c in range(N_CORES):
        e0, e1 = EXP_PER_CORE * c, EXP_PER_CORE * c + 1
        wg_c = np.concatenate(
            [np.ascontiguousarray(gate_proj[e].T) for e in (e0, e1)],
            axis=1).astype(BF16)                             # [H, 2I]
        wu_c = np.concatenate(
            [np.ascontiguousarray(up_proj[e].T) for e in (e0, e1)],
            axis=1).astype(BF16)
        wd_c = np.concatenate(
            [np.ascontiguousarray(down_proj[e].T) for e in (e0, e1)],
            axis=1).astype(BF16)                             # [I, 2H]
        sg_c = np.zeros((H, S_SLICE), BF16)
        su_c = np.zeros((H, S_SLICE), BF16)
        sd_c = np.zeros((S_SLICE, H), BF16)
        r0, r1 = c * S_SLICE_RAW, (c + 1) * S_SLICE_RAW
        sg_c[:, :S_SLICE_RAW] = shared_gate_w[r0:r1, :].T
        su_c[:, :S_SLICE_RAW] = shared_up_w[r0:r1, :].T
        sd_c[:S_SLICE_RAW, :] = shared_down_w[:, r0:r1].T
        in_maps.append({
            "xs": xsT,
            "xg": np.ascontiguousarray(
                np.concatenate([xg_all[e0], xg_all[e1]], axis=1)),
            "wg": wg_c, "wu": wu_c, "wd": wd_c,
            "sg": sg_c, "su": su_c, "sd": sd_c,
        })

    nc = _get_compiled(T, C_cap * n_rounds, w)  # n_rounds folds into capacity
    results = run_bass_kernel_spmd(nc, in_maps, core_ids=list(range(N_CORES)))

    _LAST.clear()
    _LAST.update(nc=nc, in_maps=in_maps, results=results,
                 C_cap=C_cap * n_rounds, w=w)

    # ---- host-side combine ----
    outT = np.zeros((H, T), np.float32)
    for c in range(N_CORES):
        outT += results.results[c]["ys"].astype(np.float32)
    Ccap = C_cap * n_rounds
    for c in range(N_CORES):
        ye = results.results[c]["ye"].astype(np.float32)
        for sslot in range(EXP_PER_CORE):
            e = EXP_PER_CORE * c + sslot
            cnt = counts[e]
            if cnt == 0:
                continue
            we = comb[idx_e[e], e]
            outT[:, idx_e[e]] += ye[:, sslot * Ccap:sslot * Ccap + cnt] * we[None, :]

    return np.ascontiguousarray(outT.T).reshape(B, S, H).astype(np.float32)


# revision 8
# speedup vs baseline: 1.9340x; 1.9340x over previous
"""MoE routing kernel for Trainium2 (8 NeuronCores, SPMD expert-parallel).

Contract: kernel(**full_inputs) -> full output [B, S, H] float32.

Strategy
--------
- Host: compute the (tiny) gate + group-topk routing in numpy (bit-identical
  selection to the jax reference), build the per-(token,expert) combine
  weights, and dispatch: gather each expert's tokens into a padded,
  transposed bf16 buffer.  This is the "all-to-all by topk_idx" of the
  sharding hint, done at input-sharding time.
- Device (SPMD over 8 cores): core c holds experts (2c, 2c+1) and a 1/8
  column-slice of the shared expert.  Each core runs the SwiGLU MLP for its
  two experts over their gathered tokens (unweighted), plus its shared
  slice over all tokens, producing partial outputs in [H, tokens] layout.
- Host: scale per-expert outputs by routing weights, scatter-add over
  token indices, add the 8 shared partials, transpose back.

All matmuls run in bf16 with fp32 PSUM accumulation.  Weights stream from
HBM exactly once per kernel (strided 3D DMAs put the [H, .] panels into
[128, H/128, .] SBUF tiles); token tiles stay k-resident; gate/up chains
are emitted sequentially per token-slice so PSUM slots recycle without
stalling the PE.
"""

import math

import numpy as np
import ml_dtypes

H = 2048          # hidden size
I = 1408          # intermediate per routed expert
E = 16            # routed experts
G = 4             # groups
TOPK_GROUP = 2
TOP_K = 6
N_SHARED = 2
SCALE_FACTOR = 2.5
SI = I * N_SHARED  # 2816 shared intermediate
N_CORES = 8
EXP_PER_CORE = E // N_CORES  # 2
S_SLICE_RAW = SI // N_CORES  # 352
S_SLICE = 384                # padded to 3*128
P = 128
BF16 = ml_dtypes.bfloat16

_COMPILED = {}  # (T, C_cap, w) -> nc
_LAST = {}      # debug/profiling handle for test.py


def _gate_host(hs, gate_weight, bias):
    """numpy replica of reference._gate (verified bit-identical selection)."""
    T = hs.shape[0]
    logits = hs @ gate_weight.T                       # [T, E] fp32
    scores = 1.0 / (1.0 + np.exp(-logits))
    sfc = scores + bias[None, :]
    gs = sfc.reshape(T, G, E // G)
    gsort = np.sort(gs, axis=-1)
    group_scores = gsort[..., -1] + gsort[..., -2]
    group_idx = np.argsort(-group_scores, axis=-1, kind="stable")[:, :TOPK_GROUP]
    gmask = np.zeros((T, G), bool)
    gmask[np.arange(T)[:, None], group_idx] = True
    smask = np.repeat(gmask, E // G, axis=1)
    tmp = np.where(smask, sfc, 0.0)
    topk_idx = np.argsort(-tmp, axis=-1, kind="stable")[:, :TOP_K]
    topk_w = np.take_along_axis(scores, topk_idx, axis=1)
    topk_w = topk_w / (topk_w.sum(-1, keepdims=True) + 1e-20) * SCALE_FACTOR
    return topk_idx.astype(np.int32), topk_w.astype(np.float32)


def _build(T, C_cap, w):
    """Build + compile the SPMD Bass program.

    T     : total tokens (every core sees all of them for its shared slice)
    C_cap : per-expert gathered-token capacity
    w     : routed matmul free-dim slice width; C_cap = NP_R * 2 * w
    """
    import concourse.mybir as mybir
    import concourse.tile as tile
    from concourse import bacc

    bf = mybir.dt.bfloat16
    f32 = mybir.dt.float32
    AF = mybir.ActivationFunctionType

    KH = H // P        # 16 contraction chunks over H
    MI = I // P        # 11 I chunks
    MH = H // P        # 16 output H chunks
    MS = S_SLICE // P  # 3
    NP_S = T // 1024   # shared token blocks (2 x 512 slices each)
    NP_R = C_cap // (2 * w)  # routed token blocks (2 x w slices each)
    assert C_cap == NP_R * 2 * w and w <= 512

    nc = bacc.Bacc("TRN2", target_bir_lowering=False, debug=False,
                   num_devices=N_CORES)
    xs = nc.dram_tensor("xs", [H, T], bf, kind="ExternalInput")
    xg = nc.dram_tensor("xg", [H, EXP_PER_CORE * C_cap], bf, kind="ExternalInput")
    wg = nc.dram_tensor("wg", [H, EXP_PER_CORE * I], bf, kind="ExternalInput")
    wu = nc.dram_tensor("wu", [H, EXP_PER_CORE * I], bf, kind="ExternalInput")
    wd = nc.dram_tensor("wd", [I, EXP_PER_CORE * H], bf, kind="ExternalInput")
    sg = nc.dram_tensor("sg", [H, S_SLICE], bf, kind="ExternalInput")
    su = nc.dram_tensor("su", [H, S_SLICE], bf, kind="ExternalInput")
    sd = nc.dram_tensor("sd", [S_SLICE, H], bf, kind="ExternalInput")
    ye = nc.dram_tensor("ye", [H, EXP_PER_CORE * C_cap], bf, kind="ExternalOutput")
    ys = nc.dram_tensor("ys", [H, T], bf, kind="ExternalOutput")

    # [H, C] panels viewed as [128, H/128, C] for single-DMA k-column loads
    wg_r = wg.ap().rearrange("(ko p) c -> p ko c", p=P)
    wu_r = wu.ap().rearrange("(ko p) c -> p ko c", p=P)
    wd_r = wd.ap().rearrange("(ko p) c -> p ko c", p=P)
    sg_r = sg.ap().rearrange("(ko p) c -> p ko c", p=P)
    su_r = su.ap().rearrange("(ko p) c -> p ko c", p=P)
    sd_r = sd.ap().rearrange("(ko p) c -> p ko c", p=P)

    MGS = [(0, 4), (4, 4), (8, 3)]          # I chunk groups (11)
    MGS_D = [(0, 4), (4, 4), (8, 4), (12, 4)]  # H chunk groups (16)

    with tile.TileContext(nc) as tc:
        with (
            tc.tile_pool(name="xp", bufs=34) as xp,    # x tiles <=[128,1024] bf16
            tc.tile_pool(name="wp", bufs=6) as wp,     # [128,16,128] weight cols
            tc.tile_pool(name="wdp", bufs=4) as wdp,   # [128,11,128] down cols
            tc.tile_pool(name="sdp", bufs=1) as sdp,   # [128,3,2048] shared down
            tc.tile_pool(name="itp", bufs=46) as itp,  # [128,512] bf16 inter
            tc.tile_pool(name="tmp", bufs=4) as tmp,   # silu temp
            tc.tile_pool(name="otp", bufs=6) as otp,   # [128,1024] bf16 out
            tc.tile_pool(name="pg", bufs=2, space="PSUM") as pgp,
            tc.tile_pool(name="pu", bufs=2, space="PSUM") as pup,
            tc.tile_pool(name="py", bufs=4, space="PSUM") as pyp,
        ):
            # ---------------- shared expert (column slice) ----------------
            sdt = sdp.tile([P, MS, H], bf, name="sdt", tag="sdt")
            nc.scalar.dma_start(sdt[:], sd_r[:])

            for np_ in range(NP_S):
                c0 = np_ * 1024
                xst = []
                for k in range(KH):
                    t = xp.tile([P, 1024], bf, name=f"xs{np_}_{k}", tag="x")
                    nc.scalar.dma_start(t[:], xs[k * P:(k + 1) * P, c0:c0 + 1024])
                    xst.append(t)
                sint = {}
                for m in range(MS):
                    mo = m * P
                    sgt = wp.tile([P, KH, P], bf, name=f"sgt{np_}_{m}", tag="wp")
                    nc.sync.dma_start(sgt[:], sg_r[:, :, mo:mo + P])
                    sut = wp.tile([P, KH, P], bf, name=f"sut{np_}_{m}", tag="wp")
                    nc.sync.dma_start(sut[:], su_r[:, :, mo:mo + P])
                    for j in range(2):
                        psg = pgp.tile([P, 512], f32, name=f"psgs{np_}_{m}{j}",
                                       tag="pg")
                        for k in range(KH):
                            nc.tensor.matmul(psg[:], sgt[:, k, :],
                                             xst[k][:, j * 512:(j + 1) * 512],
                                             start=(k == 0), stop=(k == KH - 1))
                        st = tmp.tile([P, 512], bf, name=f"sts{np_}_{m}{j}",
                                      tag="tmp")
                        nc.scalar.activation(st[:], psg[:], AF.Silu)
                        psu = pup.tile([P, 512], f32, name=f"psus{np_}_{m}{j}",
                                       tag="pu")
                        for k in range(KH):
                            nc.tensor.matmul(psu[:], sut[:, k, :],
                                             xst[k][:, j * 512:(j + 1) * 512],
                                             start=(k == 0), stop=(k == KH - 1))
                        it = itp.tile([P, 512], bf, name=f"si{np_}_{m}{j}",
                                      tag="it")
                        nc.vector.tensor_mul(it[:], st[:], psu[:])
                        sint[(m, j)] = it
                for M in range(MH):
                    ot = otp.tile([P, 1024], bf, name=f"ots{np_}_{M}", tag="ot")
                    for j in range(2):
                        psy = pyp.tile([P, 512], f32, name=f"psys{np_}_{M}{j}",
                                       tag="py")
                        for K in range(MS):
                            nc.tensor.matmul(psy[:], sdt[:, K, M * P:(M + 1) * P],
                                             sint[(K, j)][:],
                                             start=(K == 0), stop=(K == MS - 1))
                        nc.vector.tensor_copy(ot[:, j * 512:(j + 1) * 512], psy[:])
                    nc.gpsimd.dma_start(ys[M * P:(M + 1) * P, c0:c0 + 1024], ot[:])

            # ---------------- routed experts ----------------
            for s in range(EXP_PER_CORE):
                xgt = {}
                for np_ in range(NP_R):
                    b0 = s * C_cap + np_ * 2 * w
                    for k in range(KH):
                        t = xp.tile([P, 2 * w], bf, name=f"xg{s}_{np_}_{k}",
                                    tag="x")
                        nc.scalar.dma_start(
                            t[:], xg[k * P:(k + 1) * P, b0:b0 + 2 * w])
                        xgt[(np_, k)] = t
                inter = {}
                for m in range(MI):
                    mo = s * I + m * P
                    wgt = wp.tile([P, KH, P], bf, name=f"wgt{s}_{m}", tag="wp")
                    nc.sync.dma_start(wgt[:], wg_r[:, :, mo:mo + P])
                    wut = wp.tile([P, KH, P], bf, name=f"wut{s}_{m}", tag="wp")
                    nc.sync.dma_start(wut[:], wu_r[:, :, mo:mo + P])
                    for np_ in range(NP_R):
                        for j in range(2):
                            psg = pgp.tile([P, 512], f32,
                                           name=f"psg{s}_{m}_{np_}{j}",
                                           tag="pg")
                            for k in range(KH):
                                nc.tensor.matmul(
                                    psg[:, :w], wgt[:, k, :],
                                    xgt[(np_, k)][:, j * w:(j + 1) * w],
                                    start=(k == 0), stop=(k == KH - 1))
                            st = tmp.tile([P, 512], bf,
                                          name=f"st{s}_{m}_{np_}{j}",
                                          tag="tmp")
                            nc.scalar.activation(st[:, :w], psg[:, :w],
                                                 AF.Silu)
                            psu = pup.tile([P, 512], f32,
                                           name=f"psu{s}_{m}_{np_}{j}",
                                           tag="pu")
                            for k in range(KH):
                                nc.tensor.matmul(
                                    psu[:, :w], wut[:, k, :],
                                    xgt[(np_, k)][:, j * w:(j + 1) * w],
                                    start=(k == 0), stop=(k == KH - 1))
                            it = itp.tile([P, 512], bf,
                                          name=f"it{s}_{m}_{np_}{j}",
                                          tag="it")
                            nc.vector.tensor_mul(it[:, :w], st[:, :w],
                                                 psu[:, :w])
                            inter[(m, np_, j)] = it
                for M in range(MH):
                    Mo = s * H + M * P
                    wdt = wdp.tile([P, MI, P], bf, name=f"wdt{s}_{M}", tag="wdt")
                    nc.sync.dma_start(wdt[:], wd_r[:, :, Mo:Mo + P])
                    for np_ in range(NP_R):
                        b0 = s * C_cap + np_ * 2 * w
                        ot = otp.tile([P, 1024], bf,
                                      name=f"ot{s}_{M}_{np_}", tag="ot")
                        for j in range(2):
                            psy = pyp.tile([P, 512], f32,
                                           name=f"psy{s}_{M}_{np_}{j}",
                                           tag="py")
                            for K in range(MI):
                                nc.tensor.matmul(
                                    psy[:, :w], wdt[:, K, :],
                                    inter[(K, np_, j)][:, :w],
                                    start=(K == 0), stop=(K == MI - 1))
                            nc.vector.tensor_copy(
                                ot[:, j * w:(j + 1) * w], psy[:, :w])
                        nc.gpsimd.dma_start(
                            ye[M * P:(M + 1) * P, b0:b0 + 2 * w],
                            ot[:, :2 * w])

    nc.compile()
    return nc


def _get_compiled(T, C_cap, w):
    key = (T, C_cap, w)
    if key not in _COMPILED:
        _COMPILED[key] = _build(T, C_cap, w)
    return _COMPILED[key]


def kernel(hidden_states, gate_weight, e_score_correction_bias,
           gate_proj, up_proj, down_proj,
           shared_gate_w, shared_up_w, shared_down_w):
    from concourse.bass_utils import run_bass_kernel_spmd

    hs = np.asarray(hidden_states, dtype=np.float32)
    B, S, Hh = hs.shape
    assert Hh == H
    hsf = np.ascontiguousarray(hs.reshape(-1, H))
    T = hsf.shape[0]
    gate_weight = np.asarray(gate_weight, np.float32)
    bias = np.asarray(e_score_correction_bias, np.float32)
    gate_proj = np.asarray(gate_proj, np.float32)
    up_proj = np.asarray(up_proj, np.float32)
    down_proj = np.asarray(down_proj, np.float32)
    shared_gate_w = np.asarray(shared_gate_w, np.float32)
    shared_up_w = np.asarray(shared_up_w, np.float32)
    shared_down_w = np.asarray(shared_down_w, np.float32)

    # ---- routing on host ----
    topk_idx, topk_w = _gate_host(hsf, gate_weight, bias)
    comb = np.zeros((T, E), np.float32)
    np.add.at(comb, (np.arange(T)[:, None], topk_idx), topk_w)
    sel = np.zeros((T, E), bool)
    sel[np.arange(T)[:, None], topk_idx] = True
    idx_e = [np.nonzero(sel[:, e])[0] for e in range(E)]
    counts = np.array([len(ix) for ix in idx_e])

    maxc = max(int(counts.max()), 64)
    # slice width w (multiple of 8, <=512); capacity = NP_R * 2 * w >= maxc
    np_r = max(2, math.ceil(maxc / 2048))
    w = min(512, math.ceil(maxc / (np_r * 2 * 8)) * 8)
    C_cap = np_r * 2 * w
    assert C_cap >= maxc

    # ---- host-side dispatch (shard + transpose + bf16 cast) ----
    xsT = np.ascontiguousarray(hsf.T).astype(BF16)          # [H, T]
    xg_all = np.zeros((E, H, C_cap), BF16)
    for e in range(E):
        xg_all[e][:, :counts[e]] = xsT[:, idx_e[e]]

    in_maps = []
    for c in range(N_CORES):
        e0, e1 = EXP_PER_CORE * c, EXP_PER_CORE * c + 1
        wg_c = np.concatenate(
            [np.ascontiguousarray(gate_proj[e].T) for e in (e0, e1)],
            axis=1).astype(BF16)                             # [H, 2I]
        wu_c = np.concatenate(
            [np.ascontiguousarray(up_proj[e].T) for e in (e0, e1)],
            axis=1).astype(BF16)
        wd_c = np.concatenate(
            [np.ascontiguousarray(down_proj[e].T) for e in (e0, e1)],
            axis=1).astype(BF16)                             # [I, 2H]
        sg_c = np.zeros((H, S_SLICE), BF16)
        su_c = np.zeros((H, S_SLICE), BF16)
        sd_c = np.zeros((S_SLICE, H), BF16)
        r0, r1 = c * S_SLICE_RAW, (c + 1) * S_SLICE_RAW
        sg_c[:, :S_SLICE_RAW] = shared_gate_w[r0:r1, :].T
        su_c[:, :S_SLICE_RAW] = shared_up_w[r0:r1, :].T
        sd_c[:S_SLICE_RAW, :] = shared_down_w[:, r0:r1].T
        in_maps.append({
            "xs": xsT,
            "xg": np.ascontiguousarray(
                np.concatenate([xg_all[e0], xg_all[e1]], axis=1)),
            "wg": wg_c, "wu": wu_c, "wd": wd_c,
            "sg": sg_c, "su": su_c, "sd": sd_c,
        })

    nc = _get_compiled(T, C_cap, w)
    results = run_bass_kernel_spmd(nc, in_maps, core_ids=list(range(N_CORES)))

    _LAST.clear()
    _LAST.update(nc=nc, in_maps=in_maps, results=results, C_cap=C_cap, w=w)

    # ---- host-side combine ----
    outT = np.zeros((H, T), np.float32)
    for c in range(N_CORES):
        outT += results.results[c]["ys"].astype(np.float32)
    for c in range(N_CORES):
        ye = results.results[c]["ye"].astype(np.float32)
        for sslot in range(EXP_PER_CORE):
            e = EXP_PER_CORE * c + sslot
            cnt = counts[e]
            if cnt == 0:
                continue
            we = comb[idx_e[e], e]
            outT[:, idx_e[e]] += ye[:, sslot * C_cap:sslot * C_cap + cnt] * we[None, :]

    return np.ascontiguousarray(outT.T).reshape(B, S, H).astype(np.float32)


# revision 19
# speedup vs baseline: 1.9587x; 1.0128x over previous
"""MoE routing kernel for Trainium2 (8 NeuronCores, SPMD expert-parallel).

Contract: kernel(**full_inputs) -> full output [B, S, H] float32.

Strategy
--------
- Host: compute the (tiny) gate + group-topk routing in numpy (bit-identical
  selection to the jax reference), build the per-(token,expert) combine
  weights, and dispatch: gather each expert's tokens into a padded,
  transposed bf16 buffer.  This is the "all-to-all by topk_idx" of the
  sharding hint, done at input-sharding time.
- Device (SPMD over 8 cores): core c holds experts (2c, 2c+1) and a 1/8
  column-slice of the shared expert.  Each core runs the SwiGLU MLP for its
  two experts over their gathered tokens (unweighted), plus its shared
  slice over all tokens, producing partial outputs in [H, tokens] layout.
- Host: scale per-expert outputs by routing weights, scatter-add over
  token indices, add the 8 shared partials, transpose back.

All matmuls run in bf16 with fp32 PSUM accumulation.  Weights stream from
HBM exactly once per kernel (strided 3D DMAs put the [H, .] panels into
[128, H/128, .] SBUF tiles); token tiles stay k-resident; gate/up chains
are emitted sequentially per token-slice so PSUM slots recycle without
stalling the PE.
"""

import math

import numpy as np
import ml_dtypes

H = 2048          # hidden size
I = 1408          # intermediate per routed expert
E = 16            # routed experts
G = 4             # groups
TOPK_GROUP = 2
TOP_K = 6
N_SHARED = 2
SCALE_FACTOR = 2.5
SI = I * N_SHARED  # 2816 shared intermediate
N_CORES = 8
EXP_PER_CORE = E // N_CORES  # 2
S_SLICE_RAW = SI // N_CORES  # 352
S_SLICE = 384                # padded to 3*128
P = 128
BF16 = ml_dtypes.bfloat16

_COMPILED = {}  # (T, C_cap, w) -> nc
_LAST = {}      # debug/profiling handle for test.py


def _gate_host(hs, gate_weight, bias):
    """numpy replica of reference._gate (verified bit-identical selection)."""
    T = hs.shape[0]
    logits = hs @ gate_weight.T                       # [T, E] fp32
    scores = 1.0 / (1.0 + np.exp(-logits))
    sfc = scores + bias[None, :]
    gs = sfc.reshape(T, G, E // G)
    gsort = np.sort(gs, axis=-1)
    group_scores = gsort[..., -1] + gsort[..., -2]
    group_idx = np.argsort(-group_scores, axis=-1, kind="stable")[:, :TOPK_GROUP]
    gmask = np.zeros((T, G), bool)
    gmask[np.arange(T)[:, None], group_idx] = True
    smask = np.repeat(gmask, E // G, axis=1)
    tmp = np.where(smask, sfc, 0.0)
    topk_idx = np.argsort(-tmp, axis=-1, kind="stable")[:, :TOP_K]
    topk_w = np.take_along_axis(scores, topk_idx, axis=1)
    topk_w = topk_w / (topk_w.sum(-1, keepdims=True) + 1e-20) * SCALE_FACTOR
    return topk_idx.astype(np.int32), topk_w.astype(np.float32)


def _build(T, caps):
    """Build + compile the SPMD Bass program.

    T    : total tokens (every core sees all of them for its shared slice)
    caps : per expert slot, (C_cap, w): gathered-token capacity and matmul
           free-dim slice width; C_cap = NP_R * 2 * w
    """
    import concourse.mybir as mybir
    import concourse.tile as tile
    from concourse import bacc

    bf = mybir.dt.bfloat16
    f32 = mybir.dt.float32
    AF = mybir.ActivationFunctionType

    KH = H // P        # 16 contraction chunks over H
    MI = I // P        # 11 I chunks
    MH = H // P        # 16 output H chunks
    MS = S_SLICE // P  # 3
    NP_S = T // 1024   # shared token blocks (2 x 512 slices each)
    for (C_cap, w) in caps:
        assert C_cap % (2 * w) == 0 and w <= 512
    C_tot = sum(C_cap for C_cap, _ in caps)
    slot_base = [sum(C for C, _ in caps[:s]) for s in range(len(caps))]

    nc = bacc.Bacc("TRN2", target_bir_lowering=False, debug=False,
                   num_devices=N_CORES)
    xs = nc.dram_tensor("xs", [H, T], bf, kind="ExternalInput")
    xg = nc.dram_tensor("xg", [H, C_tot], bf, kind="ExternalInput")
    wg = nc.dram_tensor("wg", [H, EXP_PER_CORE * I], bf, kind="ExternalInput")
    wu = nc.dram_tensor("wu", [H, EXP_PER_CORE * I], bf, kind="ExternalInput")
    wd = nc.dram_tensor("wd", [I, EXP_PER_CORE * H], bf, kind="ExternalInput")
    sg = nc.dram_tensor("sg", [H, S_SLICE], bf, kind="ExternalInput")
    su = nc.dram_tensor("su", [H, S_SLICE], bf, kind="ExternalInput")
    sd = nc.dram_tensor("sd", [S_SLICE, H], bf, kind="ExternalInput")
    ye = nc.dram_tensor("ye", [H, C_tot], bf, kind="ExternalOutput")
    ys = nc.dram_tensor("ys", [H, T], bf, kind="ExternalOutput")

    # [H, C] panels viewed as [128, H/128, C] for single-DMA k-column loads
    wg_r = wg.ap().rearrange("(ko p) c -> p ko c", p=P)
    wu_r = wu.ap().rearrange("(ko p) c -> p ko c", p=P)
    wd_r = wd.ap().rearrange("(ko p) c -> p ko c", p=P)
    sg_r = sg.ap().rearrange("(ko p) c -> p ko c", p=P)
    su_r = su.ap().rearrange("(ko p) c -> p ko c", p=P)
    sd_r = sd.ap().rearrange("(ko p) c -> p ko c", p=P)

    MGS = [(0, 4), (4, 4), (8, 3)]          # I chunk groups (11)
    MGS_D = [(0, 4), (4, 4), (8, 4), (12, 4)]  # H chunk groups (16)

    with tile.TileContext(nc) as tc:
        with (
            tc.tile_pool(name="xp", bufs=34) as xp,    # x tiles <=[128,1024] bf16
            tc.tile_pool(name="wp", bufs=6) as wp,     # [128,16,128] weight cols
            tc.tile_pool(name="wdp", bufs=4) as wdp,   # [128,11,128] down cols
            tc.tile_pool(name="sdp", bufs=1) as sdp,   # [128,3,2048] shared down
            tc.tile_pool(name="itp", bufs=46) as itp,  # [128,512] bf16 inter
            tc.tile_pool(name="tmp", bufs=4) as tmp,   # silu temp
            tc.tile_pool(name="otp", bufs=6) as otp,   # [128,1024] bf16 out
            tc.tile_pool(name="pg", bufs=2, space="PSUM") as pgp,
            tc.tile_pool(name="pu", bufs=2, space="PSUM") as pup,
            tc.tile_pool(name="py", bufs=4, space="PSUM") as pyp,
        ):
            # ---------------- shared expert (column slice) ----------------
            sdt = sdp.tile([P, MS, H], bf, name="sdt", tag="sdt")
            nc.scalar.dma_start(sdt[:], sd_r[:])

            # spread the critical first block's loads over four queues so the
            # first matmul chain isn't gated on one sequencer issuing 16 DMAs
            first_engines = [nc.scalar, nc.sync, nc.gpsimd]
            for np_ in range(NP_S):
                c0 = np_ * 1024
                xst = []
                for k in range(KH):
                    t = xp.tile([P, 1024], bf, name=f"xs{np_}_{k}", tag="x")
                    eng = first_engines[k % 3] if np_ == 0 else nc.scalar
                    eng.dma_start(t[:], xs[k * P:(k + 1) * P, c0:c0 + 1024])
                    xst.append(t)
                sint = {}
                for m in range(MS):
                    mo = m * P
                    sgt = wp.tile([P, KH, P], bf, name=f"sgt{np_}_{m}", tag="wp")
                    nc.sync.dma_start(sgt[:], sg_r[:, :, mo:mo + P])
                    sut = wp.tile([P, KH, P], bf, name=f"sut{np_}_{m}", tag="wp")
                    nc.sync.dma_start(sut[:], su_r[:, :, mo:mo + P])
                    for j in range(2):
                        psg = pgp.tile([P, 512], f32, name=f"psgs{np_}_{m}{j}",
                                       tag="pg")
                        for k in range(KH):
                            nc.tensor.matmul(psg[:], sgt[:, k, :],
                                             xst[k][:, j * 512:(j + 1) * 512],
                                             start=(k == 0), stop=(k == KH - 1))
                        st = tmp.tile([P, 512], bf, name=f"sts{np_}_{m}{j}",
                                      tag="tmp")
                        nc.scalar.activation(st[:], psg[:], AF.Silu)
                        psu = pup.tile([P, 512], f32, name=f"psus{np_}_{m}{j}",
                                       tag="pu")
                        for k in range(KH):
                            nc.tensor.matmul(psu[:], sut[:, k, :],
                                             xst[k][:, j * 512:(j + 1) * 512],
                                             start=(k == 0), stop=(k == KH - 1))
                        it = itp.tile([P, 512], bf, name=f"si{np_}_{m}{j}",
                                      tag="it")
                        nc.vector.tensor_mul(it[:], st[:], psu[:])
                        sint[(m, j)] = it
                for M in range(MH):
                    ot = otp.tile([P, 1024], bf, name=f"ots{np_}_{M}", tag="ot")
                    for j in range(2):
                        psy = pyp.tile([P, 512], f32, name=f"psys{np_}_{M}{j}",
                                       tag="py")
                        for K in range(MS):
                            nc.tensor.matmul(psy[:], sdt[:, K, M * P:(M + 1) * P],
                                             sint[(K, j)][:],
                                             start=(K == 0), stop=(K == MS - 1))
                        nc.vector.tensor_copy(ot[:, j * 512:(j + 1) * 512], psy[:])
                    nc.gpsimd.dma_start(ys[M * P:(M + 1) * P, c0:c0 + 1024], ot[:])

            # ---------------- routed experts ----------------
            for s, (C_cap, w) in enumerate(caps):
                NP_R = C_cap // (2 * w)
                xgt = {}
                for np_ in range(NP_R):
                    b0 = slot_base[s] + np_ * 2 * w
                    for k in range(KH):
                        t = xp.tile([P, 2 * w], bf, name=f"xg{s}_{np_}_{k}",
                                    tag="x")
                        nc.scalar.dma_start(
                            t[:], xg[k * P:(k + 1) * P, b0:b0 + 2 * w])
                        xgt[(np_, k)] = t
                inter = {}
                for m in range(MI):
                    mo = s * I + m * P
                    wgt = wp.tile([P, KH, P], bf, name=f"wgt{s}_{m}", tag="wp")
                    nc.sync.dma_start(wgt[:], wg_r[:, :, mo:mo + P])
                    wut = wp.tile([P, KH, P], bf, name=f"wut{s}_{m}", tag="wp")
                    nc.sync.dma_start(wut[:], wu_r[:, :, mo:mo + P])
                    for np_ in range(NP_R):
                        for j in range(2):
                            psg = pgp.tile([P, 512], f32,
                                           name=f"psg{s}_{m}_{np_}{j}",
                                           tag="pg")
                            for k in range(KH):
                                nc.tensor.matmul(
                                    psg[:, :w], wgt[:, k, :],
                                    xgt[(np_, k)][:, j * w:(j + 1) * w],
                                    start=(k == 0), stop=(k == KH - 1))
                            st = tmp.tile([P, 512], bf,
                                          name=f"st{s}_{m}_{np_}{j}",
                                          tag="tmp")
                            nc.scalar.activation(st[:, :w], psg[:, :w],
                                                 AF.Silu)
                            psu = pup.tile([P, 512], f32,
                                           name=f"psu{s}_{m}_{np_}{j}",
                                           tag="pu")
                            for k in range(KH):
                                nc.tensor.matmul(
                                    psu[:, :w], wut[:, k, :],
                                    xgt[(np_, k)][:, j * w:(j + 1) * w],
                                    start=(k == 0), stop=(k == KH - 1))
                            it = itp.tile([P, 512], bf,
                                          name=f"it{s}_{m}_{np_}{j}",
                                          tag="it")
                            nc.vector.tensor_mul(it[:, :w], st[:, :w],
                                                 psu[:, :w])
                            inter[(m, np_, j)] = it
                for M in range(MH):
                    Mo = s * H + M * P
                    wdt = wdp.tile([P, MI, P], bf, name=f"wdt{s}_{M}", tag="wdt")
                    nc.sync.dma_start(wdt[:], wd_r[:, :, Mo:Mo + P])
                    for np_ in range(NP_R):
                        b0 = slot_base[s] + np_ * 2 * w
                        ot = otp.tile([P, 1024], bf,
                                      name=f"ot{s}_{M}_{np_}", tag="ot")
                        for j in range(2):
                            psy = pyp.tile([P, 512], f32,
                                           name=f"psy{s}_{M}_{np_}{j}",
                                           tag="py")
                            for K in range(MI):
                                nc.tensor.matmul(
                                    psy[:, :w], wdt[:, K, :],
                                    inter[(K, np_, j)][:, :w],
                                    start=(K == 0), stop=(K == MI - 1))
                            nc.vector.tensor_copy(
                                ot[:, j * w:(j + 1) * w], psy[:, :w])
                        nc.gpsimd.dma_start(
                            ye[M * P:(M + 1) * P, b0:b0 + 2 * w],
                            ot[:, :2 * w])

    nc.compile()
    return nc


def _get_compiled(T, caps):
    key = (T, tuple(caps))
    if key not in _COMPILED:
        _COMPILED[key] = _build(T, caps)
    return _COMPILED[key]


def _cap_for(maxc):
    maxc = max(int(maxc), 64)
    np_r = max(2, math.ceil(maxc / 2048))
    w = min(512, math.ceil(maxc / (np_r * 2 * 8)) * 8)
    C_cap = np_r * 2 * w
    assert C_cap >= maxc
    return C_cap, w


def kernel(hidden_states, gate_weight, e_score_correction_bias,
           gate_proj, up_proj, down_proj,
           shared_gate_w, shared_up_w, shared_down_w):
    from concourse.bass_utils import run_bass_kernel_spmd

    hs = np.asarray(hidden_states, dtype=np.float32)
    B, S, Hh = hs.shape
    assert Hh == H
    hsf = np.ascontiguousarray(hs.reshape(-1, H))
    T = hsf.shape[0]
    gate_weight = np.asarray(gate_weight, np.float32)
    bias = np.asarray(e_score_correction_bias, np.float32)
    gate_proj = np.asarray(gate_proj, np.float32)
    up_proj = np.asarray(up_proj, np.float32)
    down_proj = np.asarray(down_proj, np.float32)
    shared_gate_w = np.asarray(shared_gate_w, np.float32)
    shared_up_w = np.asarray(shared_up_w, np.float32)
    shared_down_w = np.asarray(shared_down_w, np.float32)

    # ---- routing on host ----
    topk_idx, topk_w = _gate_host(hsf, gate_weight, bias)
    comb = np.zeros((T, E), np.float32)
    np.add.at(comb, (np.arange(T)[:, None], topk_idx), topk_w)
    sel = np.zeros((T, E), bool)
    sel[np.arange(T)[:, None], topk_idx] = True
    idx_e = [np.nonzero(sel[:, e])[0] for e in range(E)]
    counts = np.array([len(ix) for ix in idx_e])

    # assign experts to (core, slot): slot 0 gets the 8 largest, slot 1 the
    # 8 smallest, so each slot's capacity (uniform across cores under SPMD)
    # hugs its own max count
    order = np.argsort(-counts, kind="stable")
    assign = np.zeros((N_CORES, EXP_PER_CORE), np.int64)
    for c in range(N_CORES):
        assign[c, 0] = order[c]
        assign[c, 1] = order[2 * N_CORES - 1 - c]
    caps = [
        _cap_for(counts[assign[:, 0]].max()),
        _cap_for(counts[assign[:, 1]].max()),
    ]
    slot_base = [0, caps[0][0]]
    C_tot = caps[0][0] + caps[1][0]

    # ---- host-side dispatch (shard + transpose + bf16 cast) ----
    xsT = np.ascontiguousarray(hsf.T).astype(BF16)          # [H, T]

    in_maps = []
    for c in range(N_CORES):
        e0, e1 = assign[c]
        xg_c = np.zeros((H, C_tot), BF16)
        for sslot, e in enumerate((e0, e1)):
            b0 = slot_base[sslot]
            xg_c[:, b0:b0 + counts[e]] = xsT[:, idx_e[e]]
        wg_c = np.concatenate(
            [np.ascontiguousarray(gate_proj[e].T) for e in (e0, e1)],
            axis=1).astype(BF16)                             # [H, 2I]
        wu_c = np.concatenate(
            [np.ascontiguousarray(up_proj[e].T) for e in (e0, e1)],
            axis=1).astype(BF16)
        wd_c = np.concatenate(
            [np.ascontiguousarray(down_proj[e].T) for e in (e0, e1)],
            axis=1).astype(BF16)                             # [I, 2H]
        sg_c = np.zeros((H, S_SLICE), BF16)
        su_c = np.zeros((H, S_SLICE), BF16)
        sd_c = np.zeros((S_SLICE, H), BF16)
        r0, r1 = c * S_SLICE_RAW, (c + 1) * S_SLICE_RAW
        sg_c[:, :S_SLICE_RAW] = shared_gate_w[r0:r1, :].T
        su_c[:, :S_SLICE_RAW] = shared_up_w[r0:r1, :].T
        sd_c[:S_SLICE_RAW, :] = shared_down_w[:, r0:r1].T
        in_maps.append({
            "xs": xsT, "xg": xg_c,
            "wg": wg_c, "wu": wu_c, "wd": wd_c,
            "sg": sg_c, "su": su_c, "sd": sd_c,
        })

    nc = _get_compiled(T, caps)
    results = run_bass_kernel_spmd(nc, in_maps, core_ids=list(range(N_CORES)))

    _LAST.clear()
    _LAST.update(nc=nc, in_maps=in_maps, results=results, caps=caps)

    # ---- host-side combine ----
    outT = np.zeros((H, T), np.float32)
    for c in range(N_CORES):
        outT += results.results[c]["ys"].astype(np.float32)
    for c in range(N_CORES):
        ye = results.results[c]["ye"].astype(np.float32)
        for sslot in range(EXP_PER_CORE):
            e = assign[c, sslot]
            cnt = counts[e]
            if cnt == 0:
                continue
            b0 = slot_base[sslot]
            we = comb[idx_e[e], e]
            outT[:, idx_e[e]] += ye[:, b0:b0 + cnt] * we[None, :]

    return np.ascontiguousarray(outT.T).reshape(B, S, H).astype(np.float32)
